# revision 15
# baseline (speedup 1.0000x reference)
"""Causal multi-head attention block on 8 TRN2 NeuronCores.

Sharding: tensor-parallel over heads (16 heads -> 2 per core) for QKV +
attention; AllToAll switches to token-parallel (4096 tokens -> 512 per
core) for the output projection.

Per-core device graph (all bf16 matmuls, fp32 PSUM accumulation):
  1. QKV: q^T,k^T in [e,t] layout (e = 2 heads x 64 on partitions),
     v in [t,e] layout, from a resident x^T [1024, 4096].
  2. Attention per (batch, head): scores computed TRANSPOSED
     s^T[kv, q] = k^T.T @ q^T so softmax stats land on the free axis of
     nothing -- instead the denominator comes free from a ones-column
     appended to v (rows of attn^T psum: 0:64 = numerator, 64 = denom).
     No max-subtraction (scores ~ N(0,1) after folding 1/sqrt(dk) into
     w_q on the host; exp can't overflow).
  3. Normalize via reciprocal + gpsimd partition-broadcast, stage to a
     DRAM AllToAll buffer as bf16.
  4. AllToAll (head-shards -> token-shards), then out = attn^T.T @ w_p^T
     per 512-token chunk, + host-folded constant bias vector.
"""

import numpy as np
import ml_dtypes

import concourse.bass as bass
import concourse.bacc as bacc
import concourse.mybir as mybir
from concourse.tile import TileContext
from concourse.bass_utils import run_bass_kernel_spmd

NC = 8                      # cores
B, S, D = 2, 2048, 1024
H, DK = 16, 64
HPC = H // NC               # heads per core = 2
EC = HPC * DK               # embed dims per core = 128
T = B * S                   # 4096 flattened tokens
TC = T // NC                # tokens per core chunk = 512
K8 = D // 128               # contraction tiles = 8
SCALE = 1.0 / np.sqrt(DK)

BF16 = mybir.dt.bfloat16
F32 = mybir.dt.float32
NPBF16 = ml_dtypes.bfloat16

_CACHE = {}


def _build_nc(dbg: bool = False) -> bass.Bass:
    nc = bacc.Bacc("TRN2", target_bir_lowering=False, debug=False, num_devices=NC)
    if dbg:
        dbg_q = nc.declare_dram_parameter("dbg_q", [EC, T], BF16, isOutput=True)
        dbg_k = nc.declare_dram_parameter("dbg_k", [EC, T], BF16, isOutput=True)
        dbg_v = nc.declare_dram_parameter("dbg_v", [128, T // 128 * 130], BF16, isOutput=True)
        dbg_ain = nc.declare_dram_parameter("dbg_ain", [NC, 128, 512], BF16, isOutput=True)
        dbg_aout = nc.declare_dram_parameter("dbg_aout", [NC, 128, 512], BF16, isOutput=True)

    xT = nc.declare_dram_parameter("xT", [D, T], BF16, isOutput=False)
    wqkvT = nc.declare_dram_parameter("wqkvT", [D, 3 * EC], BF16, isOutput=False)
    wpT = nc.declare_dram_parameter("wpT", [D, D], BF16, isOutput=False)
    cvec = nc.declare_dram_parameter("cvec", [1, D], F32, isOutput=False)
    out = nc.declare_dram_parameter("out", [TC, D], F32, isOutput=True)

    # [128,128] bf16 upper-tri (i<=j) mask for diagonal score tiles
    mask_np = (np.arange(128)[:, None] <= np.arange(128)[None, :]).astype(NPBF16)
    mask_dram = nc.inline_tensor(mask_np, name="mask128")

    with TileContext(nc) as tc:
        with (
            tc.tile_pool(name="const", bufs=1) as constp,
            tc.tile_pool(name="x", bufs=1) as xp,
            tc.tile_pool(name="qk", bufs=1) as qkp,
            tc.tile_pool(name="w", bufs=1) as wp,
            tc.tile_pool(name="ps", bufs=8, space="PSUM") as psp,
            tc.tile_pool(name="pt", bufs=4) as ptp,
            tc.tile_pool(name="nrm", bufs=4) as nrmp,
            tc.tile_pool(name="stage", bufs=4) as stp,
            tc.tile_pool(name="dram", bufs=1, space="DRAM") as dramp,
            tc.tile_pool(name="proj", bufs=1) as projp,
        ):
            # ---- constants ----
            # DMA-loaded tiles that feed DVE ops get "pre-touched" by a DVE
            # copy: the DVE clock absorbs the DMA-queue wait once, so the hot
            # consumers carry only their PE/ACT wait (the tensor_scalar ISA
            # struct can't encode two sync waits).
            mask_ld = constp.tile([128, 128], BF16)
            nc.sync.dma_start(out=mask_ld[:, :], in_=mask_dram[:, :])
            mask_sb = constp.tile([128, 128], BF16)
            nc.vector.tensor_copy(mask_sb[:, :], mask_ld[:, :])
            cv_ld = constp.tile([128, D], F32)
            nc.gpsimd.dma_start(out=cv_ld[:, :], in_=cvec[:, :].to_broadcast([128, D]))
            cv_b = constp.tile([128, D], F32)
            nc.vector.tensor_copy(cv_b[:, :], cv_ld[:, :])

            # ---- load x^T and weights ----
            x_sb = xp.tile([128, K8, T], BF16)          # 8 MB
            for k in range(K8):
                nc.sync.dma_start(out=x_sb[:, k, :], in_=xT[k * 128:(k + 1) * 128, :])
            wqkv_sb = wp.tile([128, K8, 3 * EC], BF16)
            for k in range(K8):
                nc.sync.dma_start(
                    out=wqkv_sb[:, k, :], in_=wqkvT[k * 128:(k + 1) * 128, :]
                )
            wp_sb = wp.tile([128, K8, D], BF16)         # w_proj^T
            for k in range(K8):
                nc.sync.dma_start(out=wp_sb[:, k, :], in_=wpT[k * 128:(k + 1) * 128, :])

            # ---- QKV projections ----
            q_sb = qkp.tile([EC, T], BF16)
            k_sb = qkp.tile([EC, T], BF16)
            # v layout: per 128-token tile, [v_h0(64) | ones | v_h1(64) | ones]
            v_sb = qkp.tile([128, T // 128, 130], BF16)
            nc.vector.memset(v_sb[:, :, 64:65], 1.0)    # ones column, head 0
            nc.vector.memset(v_sb[:, :, 129:130], 1.0)  # ones column, head 1

            NG = T // 512  # 8 groups of 512 tokens
            for sec, name in ((0, "q"), (1, "k")):
                ps = [psp.tile([128, 512], F32, tag="ps", name=f"ps{sec}_{i}") for i in range(NG)]
                for k in range(K8):
                    for n in range(NG):
                        nc.tensor.matmul(
                            ps[n][:, :],
                            lhsT=wqkv_sb[:, k, sec * EC:(sec + 1) * EC],
                            rhs=x_sb[:, k, n * 512:(n + 1) * 512],
                            start=(k == 0), stop=(k == K8 - 1),
                        )
                dst = q_sb if sec == 0 else k_sb
                for n in range(NG):
                    # b_atten is zero for this problem: plain copy (the
                    # TensorScalarPtr bias-add encoding can't hold 2 waits)
                    nc.vector.tensor_copy(
                        dst[:, n * 512:(n + 1) * 512], ps[n][:, :]
                    )

            # one PSUM bank per v token-tile: start=True clears the whole
            # bank, so accumulation groups must not share banks
            for pa in range(4):
                psv = [psp.tile([128, 128], F32, tag="ps", name=f"psv_{pa}_{i}")
                       for i in range(8)]
                for k in range(K8):
                    for j in range(8):
                        nc.tensor.matmul(
                            psv[j][:, :],
                            lhsT=x_sb[:, k, (pa * 8 + j) * 128:(pa * 8 + j + 1) * 128],
                            rhs=wqkv_sb[:, k, 2 * EC:3 * EC],
                            start=(k == 0), stop=(k == K8 - 1),
                        )
                for j in range(8):
                    # psum [128,128] -> v_sb cols {0:64 -> 0:64, 64:128 -> 65:129}
                    tt = pa * 8 + j
                    src2 = psv[j][:, :].rearrange("p (h e) -> p h e", h=2)
                    dst2 = v_sb[:, tt, :].rearrange("p (h e) -> p h e", e=65)[:, :, 0:64]
                    nc.vector.tensor_copy(dst2, src2)

            # ---- attention, 4 jobs = (b in 2) x (head in 2) ----
            a2a_in = dramp.tile([NC, 128, 512], BF16, name="a2a_in")
            for b in range(B):
                for hi in range(HPC):
                    h0 = hi * DK
                    qh = q_sb[h0:h0 + DK, b * S:(b + 1) * S]
                    kh = k_sb[h0:h0 + DK, b * S:(b + 1) * S]
                    for g in range(S // 512):           # 4 query groups
                        pso = psp.tile([128, 512], F32, tag="ps")
                        nkv = 4 * (g + 1)
                        for kj in range(nkv):
                            d = kj - 4 * g              # >=0 on diagonal band
                            j0 = 128 * d if d >= 0 else 0
                            pss = psp.tile([128, 512], F32, tag="ps")
                            nc.tensor.matmul(
                                pss[:, j0:512],
                                lhsT=kh[:, kj * 128:(kj + 1) * 128],
                                rhs=qh[:, g * 512 + j0:(g + 1) * 512],
                                start=True, stop=True,
                            )
                            pt = ptp.tile([128, 512], BF16, tag="pt")
                            nc.scalar.activation(
                                pt[:, j0:512], pss[:, j0:512],
                                mybir.ActivationFunctionType.Exp,
                            )
                            if d >= 0:
                                nc.vector.tensor_mul(
                                    pt[:, j0:j0 + 128], pt[:, j0:j0 + 128],
                                    mask_sb[:, :],
                                )
                            nc.tensor.matmul(
                                pso[:65, j0:512],
                                lhsT=v_sb[:, (b * S) // 128 + kj,
                                          hi * 65:(hi + 1) * 65],
                                rhs=pt[:, j0:512],
                                start=(kj == 0), stop=(kj == nkv - 1),
                            )
                        # normalize rows 0:64 by row 64, stage for A2A
                        dn = nrmp.tile([1, 512], F32, tag="dn")
                        nc.scalar.copy(dn[:, :], pso[64:65, :])
                        ddr = dramp.tile([1, 512], F32, tag="ddr", bufs=4,
                                         name="ddr")
                        nc.sync.dma_start(out=ddr[:, :], in_=dn[:, :])
                        dnb = nrmp.tile([64, 512], F32, tag="dnb")
                        nc.gpsimd.dma_start(
                            out=dnb[:, :], in_=ddr[:, :].to_broadcast([64, 512])
                        )
                        rb = nrmp.tile([64, 512], F32, tag="rb")
                        nc.vector.reciprocal(rb[:, :], dnb[:, :])
                        aout = stp.tile([64, 512], BF16, tag="aout")
                        nc.vector.tensor_mul(aout[:, :], pso[0:64, :], rb[:, :])
                        chunk = b * (S // 512) + g
                        nc.sync.dma_start(
                            out=a2a_in[chunk, hi * 64:(hi + 1) * 64, :],
                            in_=aout[:, :],
                        )

            if dbg:
                nc.sync.dma_start(out=dbg_q[:, :], in_=q_sb[:, :])
                nc.sync.dma_start(out=dbg_k[:, :], in_=k_sb[:, :])
                nc.sync.dma_start(out=dbg_v[:, :], in_=v_sb[:, :, :].rearrange("p a b -> p (a b)"))
                nc.sync.dma_start(out=dbg_ain[:, :, :], in_=a2a_in[:, :, :])
            a2a_out = dramp.tile([NC, 128, 512], BF16, name="a2a_out")
            nc.gpsimd.collective_compute(
                "AllToAll",
                mybir.AluOpType.bypass,
                ins=[a2a_in.opt()],
                outs=[a2a_out.opt()],
                replica_groups=[list(range(NC))],
            )

            if dbg:
                nc.sync.dma_start(out=dbg_aout[:, :, :], in_=a2a_out[:, :, :])
            # ---- output projection on my 512-token chunk ----
            at_sb = projp.tile([128, NC, 512], BF16)
            for k in range(NC):
                nc.sync.dma_start(out=at_sb[:, k, :], in_=a2a_out[k, :, :])
            for ti in range(TC // 128):
                pspj = [psp.tile([128, 512], F32, tag="ps", name=f"pspj_{ti}_{i}") for i in range(2)]
                for ng in range(2):
                    for k in range(K8):
                        nc.tensor.matmul(
                            pspj[ng][:, :],
                            lhsT=at_sb[:, k, ti * 128:(ti + 1) * 128],
                            rhs=wp_sb[:, k, ng * 512:(ng + 1) * 512],
                            start=(k == 0), stop=(k == K8 - 1),
                        )
                osb = stp.tile([128, D], F32, tag="osb")
                for ng in range(2):
                    nc.vector.tensor_add(
                        osb[:, ng * 512:(ng + 1) * 512], pspj[ng][:, :],
                        cv_b[:, ng * 512:(ng + 1) * 512],
                    )
                nc.sync.dma_start(
                    out=out[ti * 128:(ti + 1) * 128, :], in_=osb[:, :]
                )
    nc.compile()
    return nc


def _prep_inputs(x, w_atten, b_atten, w_proj, b_proj):
    x = np.asarray(x, dtype=np.float32)
    w_atten = np.asarray(w_atten, dtype=np.float32)
    b_atten = np.asarray(b_atten, dtype=np.float32)
    w_proj = np.asarray(w_proj, dtype=np.float32)
    b_proj = np.asarray(b_proj, dtype=np.float32)

    xT = np.ascontiguousarray(x.reshape(T, D).T).astype(NPBF16)
    wpT = np.ascontiguousarray(w_proj.T).astype(NPBF16)
    # v-bias routes through softmax as an additive constant: fold into cvec
    cvec = (b_atten[2 * D:3 * D] @ w_proj.T + b_proj).astype(np.float32)[None, :]

    in_maps = []
    for c in range(NC):
        r = slice(c * EC, (c + 1) * EC)
        wq = w_atten[0 * D:1 * D][r] * SCALE     # fold score scale into w_q
        wk = w_atten[1 * D:2 * D][r]
        wv = w_atten[2 * D:3 * D][r]
        wqkvT = np.ascontiguousarray(
            np.concatenate([wq.T, wk.T, wv.T], axis=1)
        ).astype(NPBF16)
        assert np.all(b_atten[:2 * D] == 0.0), "nonzero q/k bias unsupported"
        in_maps.append({
            "xT": xT, "wqkvT": wqkvT, "wpT": wpT,
            "cvec": cvec,
        })
    return in_maps


def _run(inputs: dict, trace: bool = False):
    if "nc" not in _CACHE:
        _CACHE["nc"] = _build_nc()
    nc = _CACHE["nc"]
    in_maps = _prep_inputs(**inputs)
    res = run_bass_kernel_spmd(nc, in_maps, core_ids=list(range(NC)), trace=trace)
    chunks = [res.results[c]["out"] for c in range(NC)]
    full = np.concatenate(chunks, axis=0).reshape(B, S, D).astype(np.float32)
    return full, res


def kernel(**inputs) -> np.ndarray:
    out, _ = _run(inputs, trace=False)
    return out
